# revision 20
# baseline (speedup 1.0000x reference)
"""Trainium2 Bass kernel for nn_AttentionBlock (B=8, C=1024, L=1024, H=16, G=32).

Data-parallel over batch: 8 samples -> 8 NeuronCores, one sample per core, no
collectives.  Per core, the four channel-contraction (K=1024) matmul stages
(q/k/v projections, attention-value, output projection) run in fp8e4m3 with
MatmulPerfMode.DoubleRow (two fp8 weights per PE cell -> K=256 per
instruction, ~1.8x fewer PE columns than bf16); the score matmul stays bf16
with head-pair row-group concurrency.  Softmax runs without max-subtraction:
exp(z/8 - 2) via one ACT op per s-chunk (the -2 shift keeps exp outputs
inside fp8e4's +-240 range and cancels in normalization).

  1. GroupNorm(32 groups): per-channel mean/var in ONE DVE pass per tile
     (bn_stats/bn_aggr - no ACT Square), cross-partition group reduce +
     broadcast via tiny fp32 matmuls, rsqrt via Ln/Exp (single ACT table set
     'natural_log_exp' for the whole kernel), apply as x*scale+bias into fp8
     DoubleRow-grouped tiles xb8[g][p, sub, l] = xn[256g+128sub+p, l].
     The f32 residual is NOT materialized: x tiles stay in SBUF and the
     epilogue recomputes x*scale+bias.
  2. v^T via DR matmuls (stationary = xb8 slices, moving = v weights), into
     s-chunk-PAIR-interleaved fp8 tiles vT2[scp][s, sub, h, 0:65] with an
     exact-1.0 column 64 per head that makes the attention-value matmul emit
     the softmax denominator S as PSUM row 64.
  3. Attention, per head-pair: scoresT via bf16 K=64 matmuls on PE row
     groups 0/64 (concurrent), one ACT exp per s-chunk writing fp8
     scp-interleaved exq, attention-value as fp8-DR (K=256 over s),
     normalization: copy S rows, gpsimd partition-broadcast both pars into
     one [128,512], single reciprocal, two multiplies into DR-grouped fp8
     a8 tiles.  The next pair's q/k DR projection is generator-interleaved.
  4. proj fp8-DR + (proj_beff + x*scale+bias) residual epilogue, DMA out.

All weights are preloaded at kernel start as a few large contiguous fp8 DMAs
(2 KB+ per-partition lines); v bias is folded into the proj bias on the host.
"""

import numpy as np
import ml_dtypes

import concourse.bass as bass
import concourse.bacc as bacc
import concourse.tile as tile
from concourse import mybir
from concourse.bass_utils import run_bass_kernel_spmd

F32 = mybir.dt.float32
BF16 = mybir.dt.bfloat16
F8 = mybir.dt.float8e4
DR = mybir.MatmulPerfMode.DoubleRow

B, C, L, H = 8, 1024, 1024, 16
GROUPS = 32
CH = C // H          # 64 per-head channels
EPS = 1e-5
NT = C // 128        # 8 channel tiles
LT = L // 512        # 2 free-dim chunks of 512
PAIRS = H // 2       # 8 head pairs
KG = 4               # DoubleRow contraction groups of 256 channels
SHIFT = 2.0          # exp(z/8 - SHIFT); cancels in softmax normalization


def declare_params(nc):
    p = {}
    p["x"] = nc.declare_dram_parameter("x", [C, L], F32, isOutput=False)
    p["q_w8"] = nc.declare_dram_parameter("q_w8", [128, PAIRS, KG, 2, 128],
                                          F8, isOutput=False)
    p["k_w8"] = nc.declare_dram_parameter("k_w8", [128, PAIRS, KG, 2, 128],
                                          F8, isOutput=False)
    p["v_w8"] = nc.declare_dram_parameter("v_w8", [128, KG, 2, C], F8,
                                          isOutput=False)
    p["p_w8"] = nc.declare_dram_parameter("p_w8", [128, KG, 2, C], F8,
                                          isOutput=False)
    p["q_b"] = nc.declare_dram_parameter("q_b", [128, NT], F32, isOutput=False)
    p["k_b"] = nc.declare_dram_parameter("k_b", [128, NT], F32, isOutput=False)
    p["proj_beff"] = nc.declare_dram_parameter("proj_beff", [128, NT], F32,
                                               isOutput=False)
    p["norm_w_c"] = nc.declare_dram_parameter("norm_w_c", [128, NT], F32,
                                              isOutput=False)
    p["norm_b_c"] = nc.declare_dram_parameter("norm_b_c", [128, NT], F32,
                                              isOutput=False)
    p["A_grp"] = nc.declare_dram_parameter("A_grp", [128, 4], F32,
                                           isOutput=False)
    p["A2T"] = nc.declare_dram_parameter("A2T", [4, 128], F32, isOutput=False)
    p["out"] = nc.declare_dram_parameter("out", [C, L], F32, isOutput=True)
    return p


def emit(nc, tc, ctx, params, out_handle=None):
    """Emit one whole per-core computation inside an open TileContext."""
    from contextlib import ExitStack

    x_d = params["x"]
    out_d = params["out"] if out_handle is None else out_handle
    x_ap, out_ap = x_d.ap(), out_d.ap()

    # ---- persistent pools --------------------------------------------
    consts = ctx.enter_context(tc.tile_pool(name="consts", bufs=1))
    xp = ctx.enter_context(tc.tile_pool(name="xp", bufs=NT))
    xb8_p = ctx.enter_context(tc.tile_pool(name="xb8", bufs=KG))
    wq_p = ctx.enter_context(tc.tile_pool(name="wq", bufs=1))
    vT_p = ctx.enter_context(tc.tile_pool(name="vT", bufs=KG))
    a8_p = ctx.enter_context(tc.tile_pool(name="a8", bufs=KG))
    gn_p = ctx.enter_context(tc.tile_pool(name="gn", bufs=1))

    def load_const(dram, shape, tag, eng=None):
        eng = eng or nc.scalar
        t = consts.tile(shape, F32, tag=tag, name=tag)
        eng.dma_start(out=t, in_=dram.ap())
        return t

    ag_sb = load_const(params["A_grp"], [128, 4], "ag")
    a2_sb = load_const(params["A2T"], [4, 128], "a2")
    qb_sb = load_const(params["q_b"], [128, NT], "qb")
    kb_sb = load_const(params["k_b"], [128, NT], "kb")
    pb_sb = load_const(params["proj_beff"], [128, NT], "pb")
    nw_sb = load_const(params["norm_w_c"], [128, NT], "nw")
    nb_sb = load_const(params["norm_b_c"], [128, NT], "nb")
    onesg = consts.tile([128, 2 * H], F32, tag="onesg", name="onesg")
    nc.vector.memset(onesg, 1.0)
    eps_sb = consts.tile([4, 1], F32, tag="eps", name="eps")
    nc.vector.memset(eps_sb, EPS)
    shift_sb = consts.tile([128, 1], F32, tag="shift", name="shift")
    nc.vector.memset(shift_sb, -SHIFT)

    # x tiles: 8 x [128, L] f32, alive the whole kernel (residual source)
    xt = []
    for t in range(NT):
        tt = xp.tile([128, L], F32, tag="x_t", name="x_t")
        eng = nc.sync if t % 2 == 0 else nc.gpsimd
        eng.dma_start(out=tt, in_=x_ap[t * 128:(t + 1) * 128, :])
        xt.append(tt)

    # weight preloads: vw + first-half q/k on the scalar queue (needed
    # early, ACT idle then).  Second-half q/k and proj weights are
    # DMA'd from inside phase 3 on the gpsimd queue, whose program
    # position defers their issue past the x-load window (no HBM
    # contention with the critical x DMAs).
    vw_sb = wq_p.tile([128, KG, 2, C], F8, tag="vw", name="vw")
    nc.scalar.dma_start(out=vw_sb, in_=params["v_w8"].ap())
    qw_ab = [wq_p.tile([128, 4, KG, 2, 128], F8, tag=f"qw{h}", name=f"qw{h}")
             for h in range(2)]
    kw_ab = [wq_p.tile([128, 4, KG, 2, 128], F8, tag=f"kw{h}", name=f"kw{h}")
             for h in range(2)]
    nc.scalar.dma_start(out=qw_ab[0], in_=params["q_w8"].ap()[:, 0:4])
    nc.scalar.dma_start(out=kw_ab[0], in_=params["k_w8"].ap()[:, 0:4])
    pw_sb = wq_p.tile([128, KG, 2, C], F8, tag="pw", name="pw")

    def late_weight_dmas():
        nc.gpsimd.dma_start(out=qw_ab[1], in_=params["q_w8"].ap()[:, 4:8])
        nc.gpsimd.dma_start(out=kw_ab[1], in_=params["k_w8"].ap()[:, 4:8])
        nc.gpsimd.dma_start(out=pw_sb, in_=params["p_w8"].ap())

    # ================= Phase 1: GroupNorm =============================
    xb8 = []
    for g in range(KG):
        xb8.append(xb8_p.tile([128, 2, L], F8, tag="xb8_t", name="xb8_t"))
    # a8[g][p, s, l] = normalized attention output channel 256g+128s+p
    a8 = [a8_p.tile([128, 2, L], F8, tag="a8_t", name="a8_t")
          for _ in range(KG)]

    # GroupNorm stats in TWO tile batches (0-3, 4-7) so the first xb8
    # groups are ready while the second half of x is still loading.
    scale_sb = gn_p.tile([128, NT], F32, tag="scale", name="scale")
    bias_sb = gn_p.tile([128, NT], F32, tag="bias", name="bias")
    with ExitStack() as ph1:
        ps1 = ph1.enter_context(
            tc.tile_pool(name="ps1", bufs=2, space=bass.MemorySpace.PSUM))

        stats6 = gn_p.tile([128, NT, 2, 6], F32, tag="st6", name="st6")
        mv_all = gn_p.tile([128, NT, 2], F32, tag="mva", name="mva")
        stats = gn_p.tile([128, 2 * NT], F32, tag="stats", name="stats")
        mv16 = gn_p.tile([4, 2 * NT], F32, tag="mv16", name="mv16")
        NB = NT // 2
        for b in range(2):
            ts0 = b * NB
            sl = slice(ts0, ts0 + NB)
            for t in range(ts0, ts0 + NB):
                for h2 in range(2):
                    nc.vector.bn_stats(
                        out=stats6[:, t, h2, :],
                        in_=xt[t][:, h2 * 512:(h2 + 1) * 512],
                    )
                nc.vector.bn_aggr(out=mv_all[:, t, :],
                                  in_=stats6[:, t, :, :])
            # stats: [mean_c | e2_c] per channel, e2 = var + mean^2
            sl2 = slice(NT + ts0, NT + ts0 + NB)
            nc.vector.tensor_copy(out=stats[:, sl], in_=mv_all[:, sl, 0])
            nc.vector.tensor_tensor(out=stats[:, sl2], in0=mv_all[:, sl, 0],
                                    in1=mv_all[:, sl, 0],
                                    op=mybir.AluOpType.mult)
            nc.vector.tensor_tensor(out=stats[:, sl2], in0=stats[:, sl2],
                                    in1=mv_all[:, sl, 1],
                                    op=mybir.AluOpType.add)

            # cross-partition group reduce: [4, 2NB] = A_grp^T @ stats_b
            gps = ps1.tile([4, 2 * NB], F32, tag="gps", name="gps")
            nc.tensor.matmul(gps[:, 0:NB], ag_sb, stats[:, sl])
            nc.tensor.matmul(gps[:, NB:], ag_sb, stats[:, sl2])
            inv_n = 1.0 / 32
            nc.vector.tensor_scalar_mul(out=mv16[:, sl], in0=gps[:, 0:NB],
                                        scalar1=inv_n)
            e2 = gn_p.tile([4, NB], F32, tag="e2", name="e2")
            nc.vector.tensor_scalar_mul(out=e2, in0=gps[:, NB:],
                                        scalar1=inv_n)
            m2 = gn_p.tile([4, NB], F32, tag="m2", name="m2")
            nc.vector.tensor_tensor(out=m2, in0=mv16[:, sl],
                                    in1=mv16[:, sl],
                                    op=mybir.AluOpType.mult)
            var = gn_p.tile([4, NB], F32, tag="var", name="var")
            nc.vector.tensor_tensor(out=var, in0=e2, in1=m2,
                                    op=mybir.AluOpType.subtract)
            lnv = gn_p.tile([4, NB], F32, tag="lnv", name="lnv")
            nc.scalar.activation(out=lnv, in_=var,
                                 func=mybir.ActivationFunctionType.Ln,
                                 bias=eps_sb, scale=1.0)
            # istd = exp(-0.5*ln(var+eps)); Ln/Exp share one ACT table set
            # with the softmax exp -> a single table load for the kernel.
            nc.scalar.activation(out=mv16[:, sl2], in_=lnv,
                                 func=mybir.ActivationFunctionType.Exp,
                                 scale=-0.5)

            # broadcast to channels: [128, 2NB] = A2T^T @ mv16_b
            bc = ps1.tile([128, 2 * NB], F32, tag="bc", name="bc")
            nc.tensor.matmul(bc[:, 0:NB], a2_sb, mv16[:, sl])
            nc.tensor.matmul(bc[:, NB:], a2_sb, mv16[:, sl2])

            nc.vector.tensor_tensor(out=scale_sb[:, sl], in0=nw_sb[:, sl],
                                    in1=bc[:, NB:],
                                    op=mybir.AluOpType.mult)
            tmp = gn_p.tile([128, NB], F32, tag="tmp", name="tmp")
            nc.vector.tensor_tensor(out=tmp, in0=bc[:, 0:NB],
                                    in1=scale_sb[:, sl],
                                    op=mybir.AluOpType.mult)
            nc.vector.tensor_tensor(out=bias_sb[:, sl], in0=nb_sb[:, sl],
                                    in1=tmp, op=mybir.AluOpType.subtract)

            for t in range(ts0, ts0 + NB):
                nc.vector.tensor_scalar(
                    out=xb8[t // 2][:, t % 2, :], in0=xt[t],
                    scalar1=scale_sb[:, t:t + 1], scalar2=bias_sb[:, t:t + 1],
                    op0=mybir.AluOpType.mult, op1=mybir.AluOpType.add,
                )

    # ================= Phase 2: v^T ===================================
    # vT2[scp][s, sub, h, 0:64] = v^T for s-chunk scp*2+sub; col 64 == 1.0
    vT2 = []
    for scp in range(KG):
        vt = vT_p.tile([128, 2, H, CH + 1], F8, tag="vT_t", name="vT_t")
        nc.vector.tensor_copy(
            out=vt[:, :, :, CH:CH + 1],
            in_=onesg.rearrange("p (a g o) -> p a g o", a=2, o=1))
        vT2.append(vt)

    with ExitStack() as ph2:
        vps = ph2.enter_context(
            tc.tile_pool(name="vps", bufs=NT, space=bass.MemorySpace.PSUM))
        # n=0: contraction halves split so groups 0/1 (xb8 from GN batch
        # 0) run before batch 1's xb8 exist; casts emitted per-lc right
        # after its accumulation stops (releases the bank for n=1).
        accs = [vps.tile([128, 512], F32, tag="vac", name="vac")
                for _ in range(NT)]
        for g in range(2):
            for lc in range(NT):
                nc.tensor.matmul(
                    accs[lc], xb8[g][:, :, lc * 128:(lc + 1) * 128],
                    vw_sb[:, g, :, 0:512],
                    start=(g == 0), stop=False, perf_mode=DR,
                )
        for lc in range(NT):
            for g in range(2, KG):
                nc.tensor.matmul(
                    accs[lc], xb8[g][:, :, lc * 128:(lc + 1) * 128],
                    vw_sb[:, g, :, 0:512],
                    start=False, stop=(g == KG - 1), perf_mode=DR,
                )
            nc.vector.tensor_copy(
                out=vT2[lc // 2][:, lc % 2, 0:8, 0:CH],
                in_=accs[lc].rearrange("p (h c) -> p h c", c=CH),
            )
        # n=1: lc-major with inline casts
        for lc in range(NT):
            acc = vps.tile([128, 512], F32, tag="vac", name="vac")
            for g in range(KG):
                nc.tensor.matmul(
                    acc, xb8[g][:, :, lc * 128:(lc + 1) * 128],
                    vw_sb[:, g, :, 512:1024],
                    start=(g == 0), stop=(g == KG - 1), perf_mode=DR,
                )
            nc.vector.tensor_copy(
                out=vT2[lc // 2][:, lc % 2, 8:16, 0:CH],
                in_=acc.rearrange("p (h c) -> p h c", c=CH),
            )

    # ============ Phase 3: attention with next-pair qk interleaved ====
    qk_res = {}
    with ExitStack() as ph3:
        qk_p = ph3.enter_context(tc.tile_pool(name="qk", bufs=4))
        exp_p = ph3.enter_context(tc.tile_pool(name="expp", bufs=4))
        rc_p = ph3.enter_context(tc.tile_pool(name="rcp", bufs=4))
        ssb_p = ph3.enter_context(tc.tile_pool(name="ssb", bufs=8))
        m1_p = ph3.enter_context(
            tc.tile_pool(name="m1p", bufs=2, space=bass.MemorySpace.PSUM))
        ps2_p = ph3.enter_context(
            tc.tile_pool(name="ps2p", bufs=3, space=bass.MemorySpace.PSUM))
        qkps = ph3.enter_context(
            tc.tile_pool(name="qkps", bufs=1, space=bass.MemorySpace.PSUM))

        def qk_gen(j):
            """Emit pair j's q/k DR projection in small chunks (yield
            points) for interleaving into the previous pair's stream."""
            tiles = {}
            for name, w_ab, b_sb in (("q", qw_ab, qb_sb), ("k", kw_ab, kb_sb)):
                dst = qk_p.tile([128, L], BF16, tag=f"{name}_j",
                                name=f"{name}_j")
                for n in range(LT):
                    acc = qkps.tile([128, 512], F32, tag="qka", name="qka")
                    for g in range(KG):
                        nc.tensor.matmul(
                            acc, w_ab[j // 4][:, j % 4, g, :, :],
                            xb8[g][:, :, n * 512:(n + 1) * 512],
                            start=(g == 0), stop=(g == KG - 1),
                            perf_mode=DR,
                        )
                        yield
                    nc.vector.tensor_scalar_add(
                        out=dst[:, n * 512:(n + 1) * 512], in0=acc,
                        scalar1=b_sb[:, j:j + 1],
                    )
                tiles[name] = dst
            qk_res[j] = (tiles["q"], tiles["k"])

        for _ in qk_gen(0):
            pass

        # Global 128-slot software pipeline over (pair, tcn, sc): the
        # score matmul for slot S+1 is emitted before slot S's exp (so
        # the ACT engine, which paces this phase, never starves), and
        # each DR attention-value matmul is deferred one slot past its
        # second exp.  Normalization + weight DMAs ride in pended slots.
        m1s = {}
        exqs = {}
        ps2s = {}
        pend = {}

        def emit_mm1(S):
            j, s = divmod(S, 16)
            tcn, sc = divmod(s, 8)
            q_j, k_j = qk_res[j]
            m1 = m1_p.tile([128, 2, 512], F32, tag="m1", name="m1")
            for par in range(2):
                base = CH * par
                nc.tensor.matmul(
                    m1[:, par, :],
                    k_j[base:base + CH, sc * 128:(sc + 1) * 128],
                    q_j[base:base + CH, tcn * 512:(tcn + 1) * 512],
                )
            m1s[S] = m1

        def emit_mm2(j, tcn, scd):
            if (j, tcn) not in ps2s:
                ps2s[(j, tcn)] = {
                    par: ps2_p.tile([CH + 1, 512], F32, tag="ps2",
                                    name="ps2") for par in range(2)}
            ps2 = ps2s[(j, tcn)]
            exq = exqs.pop((j, tcn, scd))
            for par in range(2):
                nc.tensor.matmul(
                    ps2[par],
                    vT2[scd][:, :, 2 * j + par, :],
                    exq[:, :, par, :],
                    start=(scd == 0), stop=(scd == KG - 1),
                    perf_mode=DR,
                )

        def normalize(j, tcn):
            ps2 = ps2s.pop((j, tcn))
            g_a, s_a = j // 2, j % 2
            for par in range(2):
                s_sb = ssb_p.tile([1, 512], F32, tag="s_sb", name="s_sb")
                nc.vector.tensor_copy(out=s_sb, in_=ps2[par][CH:CH + 1, :])
                sbb = rc_p.tile([CH, 512], F32, tag="sbb", name="sbb")
                nc.gpsimd.partition_broadcast(sbb, s_sb, channels=CH)
                rc64 = rc_p.tile([CH, 512], F32, tag="rc64", name="rc64")
                nc.vector.reciprocal_approx_fast(out=rc64, in_=sbb)
                nc.vector.tensor_tensor(
                    out=a8[g_a][CH * par:CH * (par + 1), s_a,
                                tcn * 512:(tcn + 1) * 512],
                    in0=ps2[par][0:CH, :], in1=rc64,
                    op=mybir.AluOpType.mult)

        NSLOT = PAIRS * 16
        emit_mm1(0)
        nxt = None
        for S in range(NSLOT):
            j, s = divmod(S, 16)
            tcn, sc = divmod(s, 8)
            scd, sc2 = divmod(sc, 2)
            if s == 0:
                nxt = qk_gen(j + 1) if j + 1 < PAIRS else None
            if j == 0 and s == 9:
                late_weight_dmas()
            if sc2 == 0:
                exqs[(j, tcn, scd)] = exp_p.tile([128, 2, 2, 512], F8,
                                                 tag="ex", name="ex")
            if S + 1 < NSLOT:
                emit_mm1(S + 1)
            for fn in pend.pop(S, ()):
                fn()
            nc.scalar.activation(
                out=exqs[(j, tcn, scd)][:, sc2, :, :], in_=m1s.pop(S),
                func=mybir.ActivationFunctionType.Exp,
                bias=shift_sb, scale=0.125,
            )
            if sc2 == 1:
                acts = [lambda j=j, tcn=tcn, scd=scd: emit_mm2(j, tcn, scd)]
                if scd == KG - 1:
                    acts.append(lambda j=j, tcn=tcn: normalize(j, tcn))
                pend.setdefault(S + 1, []).extend(acts)
            if nxt is not None:
                next(nxt, None)
                if 1 <= s <= 8:
                    next(nxt, None)
        for fn in pend.pop(NSLOT, ()):
            fn()

    # ================= Phase 4: proj + residual =======================
    with ExitStack() as ph4:
        out_p = ph4.enter_context(tc.tile_pool(name="outp", bufs=3))
        xr_p = ph4.enter_context(tc.tile_pool(name="xrp", bufs=3))
        ps4 = ph4.enter_context(
            tc.tile_pool(name="ps4", bufs=4, space=bass.MemorySpace.PSUM))
        for m in range(NT):
            accs = [ps4.tile([128, 512], F32, tag="p4", name="p4")
                    for _ in range(LT)]
            for g in range(KG):
                for n in range(LT):
                    nc.tensor.matmul(
                        accs[n],
                        pw_sb[:, g, :, m * 128:(m + 1) * 128],
                        a8[g][:, :, n * 512:(n + 1) * 512],
                        start=(g == 0), stop=(g == KG - 1),
                        perf_mode=DR,
                    )
            o_sb = out_p.tile([128, L], F32, tag="o_sb", name="o_sb")
            for n in range(LT):
                xres = xr_p.tile([128, 512], F32, tag="xres", name="xres")
                nc.vector.tensor_scalar(
                    out=xres, in0=xt[m][:, n * 512:(n + 1) * 512],
                    scalar1=scale_sb[:, m:m + 1], scalar2=bias_sb[:, m:m + 1],
                    op0=mybir.AluOpType.mult, op1=mybir.AluOpType.add,
                )
                nc.vector.scalar_tensor_tensor(
                    out=o_sb[:, n * 512:(n + 1) * 512], in0=accs[n],
                    scalar=pb_sb[:, m:m + 1], in1=xres,
                    op0=mybir.AluOpType.add, op1=mybir.AluOpType.add,
                )
            nc.gpsimd.dma_start(
                out=out_ap[m * 128:(m + 1) * 128, :], in_=o_sb,
            )


_CACHED = {}


def build_program(repeats=1):
    key = ("nc", repeats)
    if key in _CACHED:
        return _CACHED[key]
    from contextlib import ExitStack

    nc = bacc.Bacc("TRN2", target_bir_lowering=False, debug=False)
    with tile.TileContext(nc) as tc:
        params = declare_params(nc)
        for rep in range(repeats):
            out_h = None
            if rep > 0:
                out_h = nc.dram_tensor(f"out_scratch{rep}", [C, L], F32)
            with ExitStack() as ctx:
                emit(nc, tc, ctx, params, out_h)
    nc.compile()
    _CACHED[key] = nc
    return nc


def to_f8(a):
    return np.clip(np.asarray(a, np.float32), -240.0, 240.0).astype(
        ml_dtypes.float8_e4m3)


def host_pack(norm_w, norm_b, qkv_w, qkv_b, proj_w, proj_b):
    """Precompute packed weight layouts (all plain numpy)."""
    f = np.float32
    qkv_w = np.asarray(qkv_w, f)
    qkv_b = np.asarray(qkv_b, f)
    proj_w = np.asarray(proj_w, f)
    proj_b = np.asarray(proj_b, f)

    # q/k index packing: pair tile j holds heads 2j (cols 0:64), 2j+1
    idx_q = np.empty(C, np.int64)
    idx_k = np.empty(C, np.int64)
    for j in range(PAIRS):
        for m in range(128):
            h = 2 * j + m // CH
            i = m % CH
            idx_q[j * 128 + m] = 192 * h + i
            idx_k[j * 128 + m] = 192 * h + CH + i
    idx_v = np.empty(C, np.int64)
    for h in range(H):
        idx_v[CH * h:CH * (h + 1)] = 192 * h + 2 * CH + np.arange(CH)

    # DoubleRow packing: [p, ..., g, s, cols], contraction c = 256g+128s+p
    def pack_qk(idx):
        wT = np.ascontiguousarray(qkv_w[idx, :].T)      # [cin, 8*128]
        w = wT.reshape(KG, 2, 128, PAIRS, 128)          # [g, s, p, j, m]
        return to_f8(np.ascontiguousarray(w.transpose(2, 3, 0, 1, 4)))

    q_w8 = pack_qk(idx_q)
    k_w8 = pack_qk(idx_k)

    def pack_cc(wT):                                    # wT: [cin, cols]
        w = wT.reshape(KG, 2, 128, C)                   # [g, s, p, col]
        return to_f8(np.ascontiguousarray(w.transpose(2, 0, 1, 3)))

    v_w8 = pack_cc(np.ascontiguousarray(qkv_w[idx_v, :].T))
    p_w8 = pack_cc(np.ascontiguousarray(proj_w.T))

    q_b = np.ascontiguousarray(qkv_b[idx_q].reshape(NT, 128).T)
    k_b = np.ascontiguousarray(qkv_b[idx_k].reshape(NT, 128).T)
    # v bias passes through softmax exactly -> fold into proj bias
    pbe = proj_b + proj_w @ qkv_b[idx_v]
    proj_beff = np.ascontiguousarray(pbe.astype(f).reshape(NT, 128).T)

    norm_w_c = np.ascontiguousarray(np.asarray(norm_w, f).reshape(NT, 128).T)
    norm_b_c = np.ascontiguousarray(np.asarray(norm_b, f).reshape(NT, 128).T)

    pp = np.arange(128)
    A_grp = (pp[:, None] // 32 == np.arange(4)[None, :]).astype(f)
    A2T = np.ascontiguousarray(A_grp.T)

    return dict(
        q_w8=q_w8, k_w8=k_w8, v_w8=v_w8, p_w8=p_w8,
        q_b=q_b, k_b=k_b, proj_beff=proj_beff,
        norm_w_c=norm_w_c, norm_b_c=norm_b_c, A_grp=A_grp, A2T=A2T,
    )


def kernel(x, norm_w, norm_b, qkv_w, qkv_b, proj_w, proj_b, _trace=False):
    x = np.asarray(x, np.float32)
    shared = host_pack(norm_w, norm_b, qkv_w, qkv_b, proj_w, proj_b)
    nc = build_program()
    in_maps = [dict(shared, x=np.ascontiguousarray(x[i])) for i in range(B)]
    res = run_bass_kernel_spmd(nc, in_maps, list(range(B)), trace=_trace)
    out = np.stack([res.results[i]["out"] for i in range(B)], axis=0)
    if _trace:
        kernel._last_results = res
    return out.astype(np.float32)
